# revision 27
# baseline (speedup 1.0000x reference)
"""MultiHeadAttention Trainium2 kernel (8-core SPMD).

Problem: B=2, S=2048, DIM=1024, 16 heads, head_dim=64, fp32.
Sharding: core c -> (batch b = c//4, head-group g = c%4, 4 heads each).
Each core computes, for its batch and 4 heads:
    q = x Wq'^T            (Wq' = SCALE*Wq, no bias -- see bias algebra below)
    k = x Wk^T             (no bias)
    v = x Wv^T             (no bias)
    S^T[k,q] = k . q       (feature-major layout)
    P^T = exp(S^T) scaled per-k by m[k] = exp(SCALE * bq . k[k])
    outT[d,q] = sum_k V'[k,d] P^T[k,q]   with V' = diag(m) [V | 1]
    attn^T = outT[0:64] / outT[64]       (per-q softmax denominator)
    partial = attn^T . P_g^T             ([seq, 1024] output-projection partial)
Host sums the 4 per-group partials per batch and adds
bv @ proj_w.T + proj_b (V-bias and proj-bias commute through softmax/proj).

Bias algebra: softmax over k of SCALE*(q0+bq).(k0+bk) equals softmax of
(SCALE*q0).k0 + SCALE*bq.k0[k] -- the q0.bk and bq.bk terms are constant in k
and drop out. The per-k term is applied multiplicatively (m[k]) by scaling V
rows, and V's bias bv adds exactly bv to every attention output row.

Schedule (v2): the PE array is kept continuously streaming -- idle gaps drop
the DVFS p-state (observed 512-col matmuls at 630ns vs 270ns) and trigger
HAM 4/8 duty cycling.  Attention runs as 16 single-head units (pair, head,
qtile) of 8 chunk-groups each; the attn.V accumulation for group g is
emitted one group behind its exp so the PE never head-of-line blocks on the
ACT engine.  All remaining work (qkv projections for pair 1, V chunks,
bias-correction, output projection split into per-pair halves) is fed into
the per-group filler slots to cover the exp-paced windows.
"""

import numpy as np

import concourse.bass as bass
import concourse.mybir as mybir
import concourse.tile as tile
from concourse import bacc
from concourse import bass_utils

F32 = mybir.dt.float32
BF16 = mybir.dt.bfloat16

P = 128
DIM = 1024
S = 2048
NH = 16
DH = 64
SCALE = 1.0 / 8.0
DC = DIM // P           # 8 contraction chunks
NST = S // 512          # 4 seq tiles of 512
NCH = S // P            # 16 kpos chunks of 128
FPC = 256               # features per core (4 heads * 64)
NWARM = 4


def build_attention_bass():
    nc = bacc.Bacc(
        "TRN2",
        target_bir_lowering=False,
        debug=False,
        enable_asserts=False,
        num_devices=8,
    )
    xT = nc.dram_tensor("xT", [DIM, S], BF16, kind="ExternalInput").ap()
    wqT = nc.dram_tensor("wqT", [DIM, FPC], BF16, kind="ExternalInput").ap()
    wkT = nc.dram_tensor("wkT", [DIM, FPC], BF16, kind="ExternalInput").ap()
    wvT = nc.dram_tensor("wvT", [DIM, FPC], BF16, kind="ExternalInput").ap()
    bqz = nc.dram_tensor("bqz", [P, 2, 2], BF16, kind="ExternalInput").ap()
    pjT = nc.dram_tensor("pjT", [FPC, DIM], BF16, kind="ExternalInput").ap()
    out = nc.dram_tensor("out", [S, DIM], F32, kind="ExternalOutput").ap()

    with tile.TileContext(nc) as tc:
        _attention_body(tc, xT, wqT, wkT, wvT, bqz, pjT, out)
    nc.compile()
    return nc


def _attention_body(tc, xT, wqT, wkT, wvT, bqz, pjT, out):
    nc = tc.nc
    Exp = mybir.ActivationFunctionType.Exp
    Recip = mybir.ActivationFunctionType.Reciprocal
    Mult = mybir.AluOpType.mult
    Add = mybir.AluOpType.add

    with (
        tc.tile_pool(name="const", bufs=1) as cpool,
        tc.tile_pool(name="work", bufs=1) as wpool,
        tc.tile_pool(name="exp", bufs=6) as epool,
        tc.tile_pool(name="stage", bufs=3) as spool,
        tc.tile_pool(name="pst", bufs=2, space="PSUM") as pst,      # 2x2 banks
        tc.tile_pool(name="pavp", bufs=2, space="PSUM") as pavp,    # 2x1 bank
        tc.tile_pool(name="pfl", bufs=2, space="PSUM") as pfl,      # 2x1 bank
    ):
        # ---- input loads (order = availability priority) -----------------
        # Two HW DGE queues (sync + scalar engines), ~120GB/s each.  Only
        # the wq/wk/xt[st0] loads are issued up front; the later xt tiles
        # are issued between the lead-in matmul blocks that consume them,
        # keeping every matmul's DMA wait tight.
        wqT_r = wqT.rearrange("(dc p) f -> p dc f", p=P)
        wkT_r = wkT.rearrange("(dc p) f -> p dc f", p=P)
        wq_sb = cpool.tile([P, DC, FPC], BF16)
        nc.sync.dma_start(wq_sb[:, 0:4, :], wqT_r[:, 0:4, :])
        nc.scalar.dma_start(wq_sb[:, 4:DC, :], wqT_r[:, 4:DC, :])
        xt = cpool.tile([P, DC, S], BF16)
        xT_r = xT.rearrange("(dc p) s -> p dc s", p=P)
        nc.sync.dma_start(xt[:, 0:4, 0:512], xT_r[:, 0:4, 0:512])
        nc.scalar.dma_start(xt[:, 4:DC, 0:512], xT_r[:, 4:DC, 0:512])
        wk_sb = cpool.tile([P, DC, FPC], BF16)
        nc.sync.dma_start(wk_sb[:, 0:4, :], wkT_r[:, 0:4, :])
        nc.scalar.dma_start(wk_sb[:, 4:DC, :], wkT_r[:, 4:DC, :])

        def dma_xt(st):
            sl = slice(512 * st, 512 * (st + 1))
            nc.sync.dma_start(xt[:, 0:4, sl], xT_r[:, 0:4, sl])
            nc.scalar.dma_start(xt[:, 4:DC, sl], xT_r[:, 4:DC, sl])

        bq_sb = cpool.tile([P, 2, 2], BF16)
        wv_sb = cpool.tile([P, DC, FPC], BF16)
        pj_sb = cpool.tile([P, 2, DIM], BF16)

        q_sb = wpool.tile([P, 2, S], BF16)    # [dh-in-pair, pair, seq]
        k_sb = wpool.tile([P, 2, S], BF16)
        v_sb = wpool.tile([P, NCH, 4, 96], BF16)
        m_sb = wpool.tile([P, NCH, 4], F32)   # exp(c) per (kpos, chunk, head)
        at_sb = wpool.tile([P, 2, S], BF16)   # normalized attn^T
        pp_sb = wpool.tile([P, 32, 512], BF16)  # proj pair-0 partials

        # ---- PE warm-up during the DMA lead-in ---------------------------
        warm = wpool.tile([P, 512], BF16)
        nc.vector.memset(warm, 1.0)
        nc.vector.memset(v_sb[:, :, :, DH + 1:96], 0.0)
        wps = pavp.tile([P, 512], F32, tag="av", name="warm_ps")
        for _ in range(NWARM):
            nc.tensor.matmul(wps, lhsT=warm[:, 0:P], rhs=warm,
                             start=True, stop=True)

        # ---- helpers -----------------------------------------------------
        def cast_copy(dst, src):
            nc.vector.tensor_copy(dst, src)

        def c_and_m(p):
            # c[k] = SCALE * bq_h . k0_h[k] via block-diagonal bq operand.
            c_ps = pfl.tile([P, 512], F32, tag="fl", name=f"cps{p}")
            for ch in range(NCH):
                nc.tensor.matmul(
                    c_ps[:, 2 * ch:2 * ch + 2],
                    lhsT=k_sb[:, p, P * ch:P * (ch + 1)],
                    rhs=bq_sb[:, p, :],
                    start=True,
                    stop=True,
                )
            for h in (0, 1):
                hh = 2 * p + h
                nc.scalar.activation(
                    m_sb[:, :, hh],
                    c_ps[:, 0:2 * NCH].rearrange("p (ch h) -> p ch h", h=2)[:, :, h],
                    Exp,
                )
                # denominator column of V' is exp(c) itself
                nc.vector.tensor_copy(v_sb[:, :, hh, DH], m_sb[:, :, hh])

        def scale_v(p, ch):
            nc.vector.tensor_tensor(
                v_sb[:, ch, 2 * p:2 * p + 2, 0:DH],
                v_sb[:, ch, 2 * p:2 * p + 2, 0:DH],
                m_sb[:, ch, 2 * p:2 * p + 2, None].to_broadcast([P, 2, DH]),
                Mult,
            )

        def v_chunk(ch):
            ps = pfl.tile([P, 512], F32, tag="fl", name=f"vps{ch}")
            for dc in range(DC):
                nc.tensor.matmul(
                    ps[:, 0:FPC],
                    lhsT=xt[:, dc, P * ch:P * (ch + 1)],
                    rhs=wv_sb[:, dc, :],
                    start=(dc == 0),
                    stop=(dc == DC - 1),
                )
            nc.vector.tensor_copy(
                v_sb[:, ch, :, 0:DH],
                ps[:, 0:FPC].rearrange("p (h d) -> p h d", h=4),
            )
            scale_v(0, ch)

        def qk1_tile(wsb, st, dst):
            """pair-1 q/k projection for one seq tile (filler)."""
            ps = pfl.tile([P, 512], F32, tag="fl", name=f"qk1_{st}")
            for dc in range(DC):
                nc.tensor.matmul(
                    ps,
                    lhsT=wsb[:, dc, P:2 * P],
                    rhs=xt[:, dc, 512 * st:512 * (st + 1)],
                    start=(dc == 0),
                    stop=(dc == DC - 1),
                )
            cast_copy(dst[:, 1, 512 * st:512 * (st + 1)], ps)

        def proj0(sm, nt):
            """output projection, pair-0 half -> SBUF partial (copy on
            gpsimd to keep the DVE queue clear)."""
            ps = pfl.tile([P, 512], F32, tag="fl", name=f"p0_{sm}_{nt}")
            nc.tensor.matmul(
                ps,
                lhsT=at_sb[:, 0, P * sm:P * (sm + 1)],
                rhs=pj_sb[:, 0, 512 * nt:512 * (nt + 1)],
                start=True,
                stop=True,
            )
            nc.vector.tensor_copy(pp_sb[:, 2 * sm + nt, :], ps)

        def proj1(sm, nt):
            """output projection, pair-1 half + combine + store."""
            ps = pfl.tile([P, 512], F32, tag="fl", name=f"p1_{sm}_{nt}")
            nc.tensor.matmul(
                ps,
                lhsT=at_sb[:, 1, P * sm:P * (sm + 1)],
                rhs=pj_sb[:, 1, 512 * nt:512 * (nt + 1)],
                start=True,
                stop=True,
            )
            stg = spool.tile([P, 512], F32, tag="out", name=f"stg{sm}_{nt}")
            nc.vector.tensor_tensor(stg, pp_sb[:, 2 * sm + nt, :], ps, Add)
            eng = nc.sync if (sm + nt) % 2 == 0 else nc.scalar
            eng.dma_start(
                out[P * sm:P * (sm + 1), 512 * nt:512 * (nt + 1)], stg
            )

        # ---- lead-in: pair-0 q/k projections chasing the x DMA -----------
        # q seq-tiles fill the two "st" slots (2 banks each); k seq-tiles
        # use the "av" and "fl" slots -> all 8 PSUM banks, with emission
        # ordered so every slot's reader is emitted before its reuse.
        def q0_tile(st, qslot, j):
            sl = slice(512 * st, 512 * (st + 1))
            for dc in range(DC):
                nc.tensor.matmul(
                    qslot[:, j, :],
                    lhsT=wq_sb[:, dc, 0:P],
                    rhs=xt[:, dc, sl],
                    start=(dc == 0),
                    stop=(dc == DC - 1),
                )

        def k0_tile(st, kps):
            sl = slice(512 * st, 512 * (st + 1))
            for dc in range(DC):
                nc.tensor.matmul(
                    kps,
                    lhsT=wk_sb[:, dc, 0:P],
                    rhs=xt[:, dc, sl],
                    start=(dc == 0),
                    stop=(dc == DC - 1),
                )
            cast_copy(k_sb[:, 0, sl], kps)

        qld0 = pst.tile([P, 2, 512], F32, tag="st", name="qld0")
        q0_tile(0, qld0, 0)
        dma_xt(1)
        kl0 = pavp.tile([P, 512], F32, tag="av", name="kld0")
        k0_tile(0, kl0)
        q0_tile(1, qld0, 1)
        dma_xt(2)
        kl1 = pavp.tile([P, 512], F32, tag="av", name="kld1")
        k0_tile(1, kl1)
        cast_copy(q_sb[:, 0, 0:512], qld0[:, 0, :])
        cast_copy(q_sb[:, 0, 512:1024], qld0[:, 1, :])
        qld1 = pst.tile([P, 2, 512], F32, tag="st", name="qld1")
        q0_tile(2, qld1, 0)
        dma_xt(3)
        nc.scalar.dma_start(wv_sb, wvT.rearrange("(dc p) f -> p dc f", p=P))
        nc.sync.dma_start(bq_sb, bqz)
        kl2 = pfl.tile([P, 512], F32, tag="fl", name="kld2")
        k0_tile(2, kl2)
        q0_tile(3, qld1, 1)
        nc.sync.dma_start(pj_sb, pjT.rearrange("(c p) o -> p c o", p=P))
        kl3 = pfl.tile([P, 512], F32, tag="fl", name="kld3")
        k0_tile(3, kl3)
        cast_copy(q_sb[:, 0, 1024:1536], qld1[:, 0, :])
        cast_copy(q_sb[:, 0, 1536:2048], qld1[:, 1, :])

        c_and_m(0)
        for ch in range(6):
            v_chunk(ch)

        # ---- attention pair-units ---------------------------------------
        # Drain: the softmax reciprocal runs on DVE ([1,512], exact); the
        # broadcast runs on gpsimd; the normalize-multiply on DVE.
        def drain_head(p, e, qt, pav, direct=False, defer=False):
            # pav row 0 holds the softmax denominator.  The reciprocal of a
            # [1,512] row costs 3.3us on DVE (free-size bound), which would
            # serialize the whole DVE queue; instead stream-transpose a
            # 32-row slab so the 512 denominators land on 32 partitions
            # (16/partition), reciprocal those (~0.1us), and transpose back.
            if direct:
                # tail: skip the staging copy, read PSUM directly
                un = pav
            else:
                un = spool.tile([96, 512], F32, tag="un", bufs=4,
                                name=f"un{p}{e}{qt}")
                nc.vector.tensor_copy(un, pav[0:96, :])
            tr = spool.tile([32, 512], F32, tag="tr", bufs=2,
                            name=f"tr{p}{e}{qt}")
            nc.vector.transpose(tr, un[64:96, :])
            trv = tr.rearrange("p (j b) -> p j b", b=32)[:, :, 0]
            nc.vector.reciprocal(trv, trv)
            tr2 = spool.tile([32, 512], F32, tag="tr2", bufs=2,
                             name=f"t2{p}{e}{qt}")
            nc.vector.transpose(tr2, tr)
            rb = spool.tile([DH, 512], F32, tag="rb", bufs=4,
                            name=f"rb{p}{e}{qt}")
            nc.gpsimd.partition_broadcast(rb, tr2[0:1, :])

            def mult():
                nc.vector.tensor_tensor(
                    at_sb[DH * e:DH * (e + 1), p, 512 * qt:512 * (qt + 1)],
                    un[0:DH, :],
                    rb,
                    Mult,
                )

            if defer:
                return mult
            mult()

        def attention_unit(p, qt, fillers):
            """Both heads of pair p for one q-tile, group-interleaved so
            the PE exec queue always holds ~8 ready matmuls."""
            qsl = slice(512 * qt, 512 * (qt + 1))
            pav = [pavp.tile([P, 512], F32, tag="av", name=f"pav{p}{e}{qt}")
                   for e in (0, 1)]

            def emit_sc(e, g):
                st_t = pst.tile([P, 2, 512], F32, tag="st",
                                name=f"st{p}{e}{qt}_{g}")
                for j in (0, 1):
                    ch = 2 * g + j
                    nc.tensor.matmul(
                        st_t[:, j, :],
                        lhsT=k_sb[DH * e:DH * (e + 1), p, P * ch:P * (ch + 1)],
                        rhs=q_sb[DH * e:DH * (e + 1), p, qsl],
                        start=True,
                        stop=True,
                    )
                e_t = epool.tile([P, 2, 512], BF16, tag="e",
                                 name=f"e{p}{e}{qt}_{g}")
                nc.scalar.activation(e_t, st_t, Exp)
                return e_t

            def emit_av(e, g, e_t):
                for j in (0, 1):
                    ch = 2 * g + j
                    nc.tensor.matmul(
                        pav[e][0:96, :],
                        lhsT=v_sb[:, ch, 2 * p + e, :],
                        rhs=e_t[:, j, :],
                        start=(ch == 0),
                        stop=(ch == NCH - 1),
                    )

            prev = [None, None]
            for g in range(8):
                for e in (0, 1):
                    if fillers:
                        fillers.pop(0)()
                    e_t = emit_sc(e, g)
                    if prev[e] is not None:
                        emit_av(e, *prev[e])
                    prev[e] = (g, e_t)
            while fillers:
                fillers.pop(0)()
            last = p == 1 and qt == NST - 1
            if last:
                emit_av(0, *prev[0])
                m0 = drain_head(p, 0, qt, pav[0], direct=True, defer=True)
                emit_av(1, *prev[1])
                m1 = drain_head(p, 1, qt, pav[1], direct=True, defer=True)
                m0()
                m1()
            else:
                for e in (0, 1):
                    emit_av(e, *prev[e])
                    drain_head(p, e, qt, pav[e])

        def cm1_and_scale():
            c_and_m(1)
            for ch in range(NCH):
                scale_v(1, ch)

        def proj0_pair(sm):
            proj0(sm, 0)
            proj0(sm, 1)

        def proj1_pair(sm):
            proj1(sm, 0)
            proj1(sm, 1)

        nothing = lambda: None

        # filler thunk lists per pair-unit index (8 units: pair-0 qt 0-3,
        # then pair-1 qt 0-3).  Each unit has 16 pop slots (2 per group).
        fillers = {
            0: [lambda c=c: (v_chunk(c), v_chunk(c + 1)) for c in (6, 8, 10, 12, 14)]
               + [lambda: qk1_tile(wk_sb, 0, k_sb),
                  lambda: qk1_tile(wk_sb, 1, k_sb),
                  lambda: qk1_tile(wk_sb, 2, k_sb)],
            1: [lambda: qk1_tile(wk_sb, 3, k_sb),
                cm1_and_scale,
                lambda: qk1_tile(wq_sb, 0, q_sb)]
               + [lambda s=sm: proj0_pair(s) for sm in range(0, 4)],
            2: [lambda: qk1_tile(wq_sb, 1, q_sb)]
               + [lambda s=sm: proj0_pair(s) for sm in range(4, 8)],
            3: [lambda s=sm: proj0_pair(s) for sm in range(8, 12)],
            4: [nothing, nothing]
               + [lambda s=sm: proj0_pair(s) for sm in range(12, 16)],
            5: [lambda: qk1_tile(wq_sb, 2, q_sb), nothing]
               + [lambda s=sm: proj1_pair(s) for sm in range(0, 4)],
            6: [lambda: qk1_tile(wq_sb, 3, q_sb), nothing]
               + [lambda s=sm: proj1_pair(s) for sm in range(4, 8)],
            7: [nothing, nothing]
               + [lambda s=sm: proj1_pair(s) for sm in range(8, 12)],
        }

        ui = 0
        for p in (0, 1):
            for qt in range(NST):
                attention_unit(p, qt, fillers.get(ui, []))
                ui += 1
        for sm in range(12, 16):
            proj1_pair(sm)

# ----------------------------------------------------------------------------
# host-side wrapper
# ----------------------------------------------------------------------------

_NC_CACHE = {}


def _get_nc():
    if "nc" not in _NC_CACHE:
        _NC_CACHE["nc"] = build_attention_bass()
    return _NC_CACHE["nc"]


def make_in_maps(x, qkv_w, qkv_b, proj_w):
    """Build the 8 per-core input dicts (host-side sharding)."""
    import ml_dtypes

    bf16 = ml_dtypes.bfloat16
    in_maps = []
    for c in range(8):
        b, g = divmod(c, 4)
        fsl = slice(g * FPC, (g + 1) * FPC)
        wq = (SCALE * qkv_w[0 * DIM:1 * DIM][fsl]).T     # (1024, 256)
        wk = qkv_w[1 * DIM:2 * DIM][fsl].T
        wv = qkv_w[2 * DIM:3 * DIM][fsl].T
        bq = SCALE * qkv_b[0 * DIM:1 * DIM][fsl]         # (256,)
        bqz = np.zeros((P, 2, 2), np.float32)
        for p in range(2):
            for h in range(2):
                bqz[DH * h:DH * (h + 1), p, h] = bq[(2 * p + h) * DH:(2 * p + h + 1) * DH]
        pj = proj_w[:, fsl].T                            # (256, 1024)
        in_maps.append({
            "xT": np.ascontiguousarray(x[b].T).astype(bf16),
            "wqT": np.ascontiguousarray(wq).astype(bf16),
            "wkT": np.ascontiguousarray(wk).astype(bf16),
            "wvT": np.ascontiguousarray(wv).astype(bf16),
            "bqz": bqz.astype(bf16),
            "pjT": np.ascontiguousarray(pj).astype(bf16),
        })
    return in_maps


def combine_outputs(results, qkv_b, proj_w, proj_b):
    """Sum per-group partials and add the host-folded biases."""
    bv = qkv_b[2 * DIM:3 * DIM]
    host_bias = bv @ proj_w.T + proj_b                   # (1024,)
    out = np.empty((2, S, DIM), np.float32)
    for b in range(2):
        acc = np.zeros((S, DIM), np.float32)
        for g in range(4):
            acc += results[4 * b + g]["out"]
        out[b] = acc + host_bias[None, :]
    return out


def kernel(x, qkv_w, qkv_b, proj_w, proj_b):
    x = np.asarray(x, np.float32)
    qkv_w = np.asarray(qkv_w, np.float32)
    qkv_b = np.asarray(qkv_b, np.float32)
    proj_w = np.asarray(proj_w, np.float32)
    proj_b = np.asarray(proj_b, np.float32)

    nc = _get_nc()
    in_maps = make_in_maps(x, qkv_w, qkv_b, proj_w)
    res = bass_utils.run_bass_kernel_spmd(nc, in_maps, core_ids=list(range(8)))
    return combine_outputs(res.results, qkv_b, proj_w, proj_b)
